# revision 23
# baseline (speedup 1.0000x reference)
import sys

for _p in ("/opt/trn_rl_repo", "/root/.axon_site/_ro/trn_rl_repo"):
    if _p not in sys.path:
        sys.path.insert(0, _p)

import ml_dtypes
import numpy as np

import concourse.bass as bass
import concourse.mybir as mybir
from concourse import masks, tile
from concourse.bass_utils import run_bass_kernel_spmd
from concourse.vector_clock import ScopedClock

F32 = mybir.dt.float32
F32R = mybir.dt.float32r
BF16 = mybir.dt.bfloat16
AF = mybir.ActivationFunctionType
ALU = mybir.AluOpType

B, N, D, H, HD = 4, 2048, 256, 4, 64
NEG_SLOPE = 0.2
P = 128
NI = N // 2
NT = N // P
KT = D // P
JT = NT
ISUB = NI // P
NCORES = 8
WC = D + 2 * H
HP = H * (HD + 1)


def _patch_tile_drain():
    if getattr(tile.TileContext, "_drain_patched", False):
        return

    def _drain_and_barrier(self, tick_clock, wait_clock):
        nc = self.nc
        drain_inst = nc.sync.drain()
        wait_clock.add_sem_waits(
            drain_inst.ins, ScopedClock({None: tick_clock.global_clock})
        )
        si = drain_inst.ins.sync_info
        waits = list(si.on_wait) if (si and si.on_wait) else []
        if len(waits) > 1:
            ups = list(si.on_update) if (si and si.on_update) else []
            drain_inst.ins.sync_info = mybir.SyncInfo(on_wait=waits[:1], on_update=ups)
            for i in range(1, len(waits)):
                extra = nc.sync.drain()
                extra.ins.sync_info = mybir.SyncInfo(
                    on_wait=waits[i : i + 1], on_update=[]
                )
        nc.all_engine_barrier()
        assert self.sems is not None
        popped = nc._tile_sem_poison_stack.pop()
        assert popped is self._sem_poison
        nc.clear_and_free_semaphores(list(self.sems.allocated().values()))
        nc.all_engine_barrier()

    tile.TileContext._drain_and_barrier = _drain_and_barrier
    tile.TileContext._drain_patched = True


def _split_waits(nc, maxw=1):
    n_split = 0
    for f in nc.m.functions:
        for bb in f.blocks:
            insts = list(bb.instructions)
            out = []
            changed = False
            for inst in insts:
                si = inst.sync_info
                waits = list(si.on_wait) if (si and si.on_wait) else []
                if len(waits) > maxw and inst.engine is not None:
                    changed = True
                    extra, keep = waits[:-maxw], waits[-maxw:]
                    for k in range(0, len(extra), maxw):
                        d = mybir.InstEventSemaphore(
                            name=f"{inst.name}-wsplit{k}", ins=[], outs=[]
                        )
                        d.engine = inst.engine
                        d.sync_info = mybir.SyncInfo(
                            on_wait=extra[k : k + maxw], on_update=[]
                        )
                        out.append(d)
                        n_split += 1
                    ups = list(si.on_update) if (si and si.on_update) else []
                    inst.sync_info = mybir.SyncInfo(on_wait=keep, on_update=ups)
                out.append(inst)
            if changed:
                bb.instructions = out
    return n_split


def build_nc():
    _patch_tile_drain()
    nc = bass.Bass("TRN2", target_bir_lowering=False, debug=False)

    xbt = nc.dram_tensor("xbt", [D, N], F32, kind="ExternalInput")
    xit = nc.dram_tensor("xit", [D, NI], F32, kind="ExternalInput")
    wta = nc.dram_tensor("wta", [D, WC], F32, kind="ExternalInput")
    adjtb = nc.dram_tensor("adjtb", [N, NI], BF16, kind="ExternalInput")
    selm = nc.dram_tensor("selm", [H, H * P], BF16, kind="ExternalInput")
    outs = nc.dram_tensor("outs", [NI, D], F32, kind="ExternalOutput")

    with tile.TileContext(nc) as tc:
        with (
            tc.tile_pool(name="const", bufs=1) as constp,
            tc.tile_pool(name="big", bufs=1) as bigp,
            tc.tile_pool(name="rows", bufs=1) as rowsp,
            tc.tile_pool(name="adjt", bufs=17) as adjtp,
            tc.tile_pool(name="vwork", bufs=6) as vp,
            tc.tile_pool(name="ptwork", bufs=21) as ptp,
            tc.tile_pool(name="ostage", bufs=9) as ostagep,
            tc.tile_pool(name="small", bufs=8) as smallp,
            tc.tile_pool(name="psall", bufs=1, space="PSUM") as psall,
        ):
            ps_ctr = [0]

            def ps_tile(shape, name, tag=None):
                if tag is None:
                    tag = f"bank{ps_ctr[0] % 8}"
                    ps_ctr[0] += 1
                return psall.tile(shape, F32, tag=tag, name=name)

            pe_prev = [None]

            def pe(bi):
                if pe_prev[0] is not None:
                    tile.add_dep_helper(bi.ins, pe_prev[0], reason="pe-order")
                pe_prev[0] = bi.ins
                return bi

            ident = constp.tile([P, P], F32, tag="ident")
            masks.make_identity(nc, ident[:])
            ones1 = constp.tile([1, P], BF16, tag="ones1")
            nc.vector.memset(ones1[:], 1.0)

            wta_sb = [
                constp.tile([P, WC], F32, tag=f"wta{kt}", name=f"wta_sb{kt}")
                for kt in range(KT)
            ]
            wta_r = [
                constp.tile([P, WC], F32R, tag=f"wtar{kt}", name=f"wta_r{kt}")
                for kt in range(KT)
            ]
            sel_sb = constp.tile([H, H * P], BF16, tag="selm")
            nc.sync.dma_start(sel_sb[:], selm[:])
            sels = [sel_sb[:, h * P : (h + 1) * P] for h in range(H)]
            for kt in range(KT):
                nc.sync.dma_start(wta_sb[kt][:], wta[kt * P : (kt + 1) * P, :])
                nc.scalar.activation(wta_r[kt][:], wta_sb[kt][:], AF.Copy)

            xt_raw = bigp.tile([P, KT * N], F32, tag="xtraw")
            xit_raw = bigp.tile([P, KT * NI], F32, tag="xitraw")
            xt_sb = bigp.tile([P, KT * N], F32R, tag="xt")
            xit_sb = bigp.tile([P, KT * NI], F32R, tag="xit")
            for kt in range(KT):
                nc.sync.dma_start(
                    xit_raw[:, kt * NI : (kt + 1) * NI], xit[kt * P : (kt + 1) * P, :]
                )
                nc.scalar.activation(
                    xit_sb[:, kt * NI : (kt + 1) * NI],
                    xit_raw[:, kt * NI : (kt + 1) * NI],
                    AF.Copy,
                )
            for kt in range(KT):
                nc.sync.dma_start(
                    xt_raw[:, kt * N : (kt + 1) * N], xbt[kt * P : (kt + 1) * P, :]
                )

            adjts = []
            for jt in range(JT):
                adjt = adjtp.tile([P, NI], BF16, tag="adjt", name=f"adjt_{jt}")
                nc.sync.dma_start(adjt[:], adjtb[jt * P : (jt + 1) * P, :])
                adjts.append(adjt)

            er4 = rowsp.tile([H, NI], BF16, tag="er4")
            for c in range(NI // 512):
                pss = ps_tile([H, 512], f"pss_{c}")
                for kt in range(KT):
                    pe(nc.tensor.matmul(
                        pss[:],
                        wta_r[kt][:, D : D + H],
                        xit_sb[:, kt * NI + c * 512 : kt * NI + (c + 1) * 512],
                        start=(kt == 0),
                        stop=(kt == KT - 1),
                    ))
                nc.scalar.activation(
                    er4[:, c * 512 : (c + 1) * 512],
                    pss[:],
                    AF.Exp,
                    scale=-(1.0 - NEG_SLOPE),
                )
            e2rep = bigp.tile([P, H * NI], BF16, tag="e2rep")

            def emit_e2rep(h):
                for c in range(NI // 512):
                    psb = ps_tile([P, 512], f"psb_{h}_{c}")
                    pe(nc.tensor.matmul(
                        psb[:], sels[h], er4[0:H, c * 512 : (c + 1) * 512]
                    ))
                    nc.scalar.activation(
                        e2rep[:, h * NI + c * 512 : h * NI + (c + 1) * 512],
                        psb[:],
                        AF.Copy,
                    )

            emit_e2rep(0)

            hplus = bigp.tile([P, NT * HP], BF16, tag="hplus")
            nc.gpsimd.memset(hplus[:], 1.0)
            f1_sb = bigp.tile([P, NT * H], F32, tag="f1")
            g_sb = bigp.tile([P, NT * H], F32, tag="g")
            for nt in range(NT):
                if nt in (2, 4, 6):
                    emit_e2rep(nt // 2)
                if nt % 4 == 0:
                    chunk = nt // 4
                    for kt in range(KT):
                        lo = kt * N + chunk * 512
                        nc.scalar.activation(
                            xt_sb[:, lo : lo + 512],
                            xt_raw[:, lo : lo + 512],
                            AF.Copy,
                        )
                psh = ps_tile([P, WC], f"psh_{nt}")
                for kt in range(KT):
                    pe(nc.tensor.matmul(
                        psh[:],
                        xt_sb[:, kt * N + nt * P : kt * N + (nt + 1) * P],
                        wta_r[kt][:],
                        start=(kt == 0),
                        stop=(kt == KT - 1),
                    ))
                nc.scalar.activation(
                    f1_sb[:, nt * H : (nt + 1) * H],
                    psh[:, D + H : D + 2 * H],
                    AF.Exp,
                )
                nc.scalar.activation(
                    g_sb[:, nt * H : (nt + 1) * H],
                    psh[:, D + H : D + 2 * H],
                    AF.Exp,
                    scale=-(1.0 - NEG_SLOPE),
                )
                for h in range(H):
                    nc.scalar.activation(
                        hplus[:, nt * HP + h * (HD + 1) : nt * HP + h * (HD + 1) + HD],
                        psh[:, h * HD : (h + 1) * HD],
                        AF.Copy,
                        scale=f1_sb[:, nt * H + h : nt * H + h + 1],
                    )

            hp_ones = hplus[:].rearrange("p (t h c) -> p t h c", t=NT, h=H)
            nc.scalar.activation(
                hp_ones[:, :, :, HD : HD + 1],
                f1_sb[:].rearrange("p (t h) -> p t h", t=NT)[:, :, :, None],
                AF.Copy,
            )

            psoT = [
                ps_tile([HD + 1, 512], f"psoT_{hh}", tag=f"bank{hh}")
                for hh in range(2 * H)
            ]
            ost_tiles = [
                ostagep.tile([P, D], F32, tag="ost", name=f"ost_{q}")
                for q in range(ISUB)
            ]
            for h in range(H):
                pts = []
                for jt in range(JT):
                    v = vp.tile([P, NI], BF16, tag="v")
                    nc.vector.tensor_scalar(
                        v[:],
                        e2rep[:, h * NI : (h + 1) * NI],
                        g_sb[:, jt * H + h : jt * H + h + 1],
                        1.0,
                        ALU.mult,
                        ALU.max,
                    )
                    pt = ptp.tile([P, NI], BF16, tag="pt", name=f"pt_{h}_{jt}")
                    nc.vector.tensor_tensor(pt[:], v[:], adjts[jt][:], ALU.mult)
                    pts.append(pt)
                for half in range(2):
                    for jt in range(JT):
                        pe(nc.tensor.matmul(
                            psoT[h * 2 + half][:],
                            hplus[
                                :,
                                jt * HP + h * (HD + 1) : jt * HP + (h + 1) * (HD + 1),
                            ],
                            pts[jt][:, half * 512 : (half + 1) * 512],
                            start=(jt == 0),
                            stop=(jt == JT - 1),
                            skip_group_check=True,
                        ))
                for half in range(2):
                    soT = ostagep.tile(
                        [HD + 1, 512], F32, tag="soT", name=f"soT_{h}_{half}"
                    )
                    nc.scalar.activation(soT[:], psoT[h * 2 + half][:], AF.Copy)
                    for q in range(4):
                        isub = half * 4 + q
                        ps2 = ps_tile(
                            [P, HD + 1], f"ps2_{h}_{isub}", tag=f"bank{h * 2 + half}"
                        )
                        pe(nc.tensor.transpose(
                            ps2[:],
                            soT[:, q * P : (q + 1) * P],
                            ident[0 : HD + 1, 0 : HD + 1],
                        ))
                        rec = smallp.tile([P, 1], F32, tag="rec")
                        nc.vector.reciprocal(rec[:], ps2[:, HD : HD + 1])
                        nc.scalar.activation(
                            ost_tiles[isub][:, h * HD : (h + 1) * HD],
                            ps2[:, 0:HD],
                            AF.Copy,
                            scale=rec[:],
                        )
            for isub in range(ISUB):
                nc.sync.dma_start(
                    outs[isub * P : (isub + 1) * P, :], ost_tiles[isub][:]
                )

    _split_waits(nc)
    nc.finalize()
    return nc


_NC_CACHE = None


def _get_nc():
    global _NC_CACHE
    if _NC_CACHE is None:
        _NC_CACHE = build_nc()
    return _NC_CACHE


def make_in_maps(x, adj, W, a_src, a_dst):
    x = np.ascontiguousarray(x, dtype=np.float32)
    W = np.ascontiguousarray(W, dtype=np.float32)
    a_src = np.ascontiguousarray(a_src, dtype=np.float32)
    a_dst = np.ascontiguousarray(a_dst, dtype=np.float32)

    A_src = np.zeros((D, H), np.float32)
    A_dst = np.zeros((D, H), np.float32)
    for h in range(H):
        A_src[h * HD : (h + 1) * HD, h] = a_src[h]
        A_dst[h * HD : (h + 1) * HD, h] = a_dst[h]
    Wt = W.T.astype(np.float32)
    wta = np.ascontiguousarray(
        np.concatenate([Wt, Wt @ A_src, Wt @ A_dst], axis=1), dtype=np.float32
    )

    selm = np.zeros((H, H * P), ml_dtypes.bfloat16)
    for h in range(H):
        selm[h, h * P : (h + 1) * P] = 1.0
    in_maps = []
    adjT_cache = {}
    for c in range(NCORES):
        b, ihalf = c // 2, c % 2
        ilo = ihalf * NI
        if b not in adjT_cache:
            adjT_cache[b] = adj[b].astype(ml_dtypes.bfloat16).T
        in_maps.append(
            {
                "xbt": np.ascontiguousarray(x[b].T),
                "xit": np.ascontiguousarray(x[b, ilo : ilo + NI, :].T),
                "wta": wta,
                "adjtb": np.ascontiguousarray(adjT_cache[b][:, ilo : ilo + NI]),
                "selm": selm,
            }
        )
    return in_maps


def kernel(x, adj, W, a_src, a_dst):
    in_maps = make_in_maps(x, adj, W, a_src, a_dst)
    nc = _get_nc()
    res = run_bass_kernel_spmd(nc, in_maps, list(range(NCORES)))

    out = np.empty((B, N, D), np.float32)
    for c in range(NCORES):
        b, ihalf = c // 2, c % 2
        ilo = ihalf * NI
        out[b, ilo : ilo + NI, :] = res.results[c]["outs"]
    return out
